# revision 9
# baseline (speedup 1.0000x reference)
"""Trainium2 Bass kernel for nn_CustomGate: y = (I_64 (x) M (x) I_64) @ x.

Math: viewing x as (a=64, j=64, r=64, b=128), the gate is
    y[a,i,r,b] = sum_j M[i,j] * x[a,j,r,b]      (complex, M is 64x64)

Complex arithmetic is folded into one real 128x128 stationary weight
    W = [[Mr^T,  Mi^T ],
         [-Mi^T, Mr^T ]]           (W[p,i] layout, p = contraction)
with rhs columns stacked as [x_real(j); x_imag(j)], so out = W.T @ rhs
gives [y_real(i); y_imag(i)] per column. Each core owns 8 of the 64
`a` values; its 8*8192 = 65536 columns are laid out p-major on the
host as one [128, 65536] block, so the kernel is a pure streaming
128x128 GEMM with free chunking.

Precision: the correctness gate is rel_err < 2e-2; fp16 I/O gives
~3.6e-4 while HALVING the HBM/DMA traffic vs fp32. The kernel is
DMA-bound (~33.5 MB/core at ~425 GB/s ~= 79 us), so every other stage
is given slack and the two HWDGE rings are kept saturated end-to-end:
  * the WHOLE output (131 KiB/partition) lives in SBUF, so copies
    never wait on store completion and loads never stall on output
    buffers; stores are 2 MiB (16 KiB/partition descriptors = peak
    efficiency) and fire as soon as their columns are drained;
  * matmuls are fp16 (1 PE pass, not 4); a ~3.4 us warmup burst of
    dummy matmuls on a zero pad of the weight tile trips the PE HAM
    activity monitor to 2.4 GHz before real data arrives;
  * PSUM is drained with 1024-col copies spanning two PSUM banks,
    alternating between the Vector and Scalar engines;
  * input loads ride the Sync HWDGE ring, stores the Scalar ring, and
    the tail stores move to the by-then-idle Sync ring.
"""

import numpy as np

import concourse.bacc as bacc
import concourse.mybir as mybir
import concourse.tile as tile
from concourse.bass_utils import run_bass_kernel_spmd

DIM = 64
WIRES = 3
BATCH = 128
D = DIM**WIRES          # 262144
N_CORES = 8
A_PER_CORE = DIM // N_CORES
FREE = DIM * BATCH              # 8192 columns per a-slice
COLS = A_PER_CORE * FREE        # 65536 columns per core
P = 128
MM_N = 512              # moving-operand max per matmul
CP_N = 1024             # PSUM-drain copy width (2 banks)
WPAD = 512              # zero pad after W for PE warmup matmuls

# chunk schedule: small chunks while the pipeline fills/drains, 1 MiB
# (4096 cols) in the middle. Each chunk is loaded AND stored as one
# DMA; loads and stores alternate across the two HWDGE rings (a single
# ring sustains only ~340 GB/s; both together ~420).
CHUNKS = [512, 512, 1024, 2048] + [4096] * 14 + [2048, 1024, 512, 512]
STORE_LAG = 2           # store chunk j fires after chunk j+2's compute
assert sum(CHUNKS) == COLS

_cached = {}


def _build_nc():
    f16 = mybir.dt.float16
    f32 = mybir.dt.float32
    nc = bacc.Bacc("TRN2", target_bir_lowering=False, debug=False,
                   num_devices=N_CORES)
    xs = nc.dram_tensor("xs", [P, COLS], f16, kind="ExternalInput").ap()
    w = nc.dram_tensor("w", [P, P + WPAD], f16, kind="ExternalInput").ap()
    ys = nc.dram_tensor("ys", [P, COLS], f16, kind="ExternalOutput").ap()

    with tile.TileContext(nc) as tc:
        with (
            tc.tile_pool(name="wpool", bufs=1) as wpool,
            tc.tile_pool(name="inpool", bufs=6) as inpool,
            tc.tile_pool(name="ypool", bufs=1) as ypool,
            tc.tile_pool(name="pspool", bufs=4, space="PSUM") as pspool,
        ):
            wt = wpool.tile([P, P + WPAD], f16)
            # weight load off the Sync engine so the first bulk input
            # DMA issues as early as possible
            nc.gpsimd.dma_start(wt[:], w[:, :])

            # whole-output SBUF buffer: 131 KiB/partition
            yt = ypool.tile([P, COLS], f16)

            # ~3.4 us of dummy matmuls on the zero pad: trips the HAM
            # activity window so the PE is at 2.4 GHz when real chunks
            # arrive (idle/cold default is 1.2 GHz).
            for i in range(8):
                psw = pspool.tile([P, CP_N], f32, tag="ps")
                nc.tensor.matmul(psw[:, :MM_N], wt[:, :P],
                                 wt[:, P:P + MM_N], start=True, stop=True)

            offs = []
            f0 = 0
            for fch in CHUNKS:
                offs.append((f0, fch))
                f0 += fch

            def store(j):
                s0_, sw_ = offs[j]
                # store j rides the opposite ring from load j, so each
                # ring carries one load + one store per chunk pair
                eng = nc.scalar if j % 2 == 0 else nc.sync
                eng.dma_start(ys[:, s0_:s0_ + sw_], yt[:, s0_:s0_ + sw_])

            ncopy = 0
            for j, (f0, fch) in enumerate(offs):
                xt = inpool.tile([P, fch], f16, tag="xt")
                eng = nc.sync if j % 2 == 0 else nc.scalar
                eng.dma_start(xt[:], xs[:, f0:f0 + fch])
                for c0 in range(0, fch, CP_N):
                    cw = min(CP_N, fch - c0)
                    ps = pspool.tile([P, CP_N], f32, tag="ps")
                    for k0 in range(0, cw, MM_N):
                        nc.tensor.matmul(
                            ps[:, k0:k0 + MM_N], wt[:, :P],
                            xt[:, c0 + k0:c0 + k0 + MM_N],
                            start=True, stop=True)
                    dst = yt[:, f0 + c0:f0 + c0 + cw]
                    if ncopy % 2:
                        nc.scalar.copy(dst, ps[:, :cw])
                    else:
                        nc.vector.tensor_copy(dst, ps[:, :cw])
                    ncopy += 1
                if j >= STORE_LAG:
                    store(j - STORE_LAG)
            for j in range(len(offs) - STORE_LAG, len(offs)):
                store(j)

    nc.compile()
    return nc


def _get_nc():
    if "nc" not in _cached:
        _cached["nc"] = _build_nc()
    return _cached["nc"]


def kernel(M_real, M_imag, x_real, x_imag, **run_kwargs):
    M_real = np.ascontiguousarray(np.asarray(M_real, dtype=np.float32))
    M_imag = np.ascontiguousarray(np.asarray(M_imag, dtype=np.float32))
    x_real = np.asarray(x_real, dtype=np.float32)
    x_imag = np.asarray(x_imag, dtype=np.float32)

    # Stationary weight W[p, i] (see module docstring) + zero warmup pad.
    W = np.zeros((P, P + WPAD), dtype=np.float16)
    W[:, :P] = np.block([[M_real.T, M_imag.T],
                         [-M_imag.T, M_real.T]]).astype(np.float16)

    # Per-core p-major layout: xs[core, p, a_local*8192 + f] with
    # p = j for real, 64+j for imag; column index = (a_local, r, b).
    xr = x_real.reshape(N_CORES, A_PER_CORE, DIM, FREE)
    xi = x_imag.reshape(N_CORES, A_PER_CORE, DIM, FREE)
    xs_all = np.empty((N_CORES, P, COLS), dtype=np.float16)
    xs_all[:, :DIM, :] = xr.transpose(0, 2, 1, 3).reshape(N_CORES, DIM, COLS)
    xs_all[:, DIM:, :] = xi.transpose(0, 2, 1, 3).reshape(N_CORES, DIM, COLS)

    nc = _get_nc()
    in_maps = [{"xs": xs_all[c], "w": W} for c in range(N_CORES)]
    r = run_bass_kernel_spmd(nc, in_maps, list(range(N_CORES)), **run_kwargs)
    if run_kwargs:
        _cached["last_result"] = r
    results = r.results

    ys_all = np.stack([results[c]["ys"] for c in range(N_CORES)])
    ys_all = ys_all.reshape(N_CORES, P, A_PER_CORE, FREE)
    y_real = (ys_all[:, :DIM].transpose(0, 2, 1, 3)
              .reshape(D, BATCH).astype(np.float32))
    y_imag = (ys_all[:, DIM:].transpose(0, 2, 1, 3)
              .reshape(D, BATCH).astype(np.float32))
    return (y_real + 1j * y_imag).astype(np.complex64)
